# revision 28
# baseline (speedup 1.0000x reference)
"""MoE (top-2 of 8 experts) Trainium2 kernel — 8 NeuronCores.

Strategy (per sharding hint: expert parallelism + dispatch-by-routed-expert):
  Launch A (data-parallel gate): each core computes fp32 router
    logits/softmax/top-2 for its 1024-token shard on device, emitting the
    probs output plus per-token (top1, top2, w1, w2) routing records.
  Host dispatch: the device-computed routing records are reshaped into
    per-expert token index lists (pure index logistics — the all-to-all
    "sharding" step), padded to a static capacity.
  Launch B (expert-parallel MLP): core e owns expert e. It dma_gather's
    its routed tokens' rows from a replicated bf16 copy of x (transposed
    on the fly into [D, tokens] matmul layout), runs
    gelu(x@w1+b1)@w2 in bf16 with fp32 accumulation, scales rows by the
    renormalized gate weight, and writes the dense per-expert result.
  Host combine: out[token] += y_expert[slot] + w*b2 (each (token, rank)
    pair is owned by exactly one expert core), probs shards concatenated.

reps>1 builds a benchmark variant: the full body (input DMAs included)
re-executes in a hardware loop so per-iteration wall-clock deltas measure
the kernel without the ~90 ms axon dispatch floor.
"""

import numpy as np
import ml_dtypes

import concourse.bacc as bacc
import concourse.mybir as mybir
import concourse.tile as tile
from concourse import bass_utils

F32 = mybir.dt.float32
BF16 = mybir.dt.bfloat16
I16 = mybir.dt.int16
AF = mybir.ActivationFunctionType
ALU = mybir.AluOpType
AX = mybir.AxisListType

E, D, H, K = 8, 512, 1024, 2
B, S = 4, 2048
N = B * S          # 8192 tokens
NC = 8             # cores
NSH = N // NC      # tokens per core in the gate launch
GT = NSH // 128    # token tiles per core in the gate launch (8)

_CACHE = {}


def _build_gate_nc(reps=1):
    """Launch A: fp32 gate + softmax + top-2 for a 1024-token shard."""
    nc = bacc.Bacc("TRN2", target_bir_lowering=False, debug=False, num_devices=NC)
    xt = nc.dram_tensor("xt", [D, NSH], F32, kind="ExternalInput")
    gw = nc.dram_tensor("gw", [D, E], F32, kind="ExternalInput")
    gb = nc.dram_tensor("gb", [1, E], F32, kind="ExternalInput")
    iota = nc.dram_tensor("iota", [128, E], F32, kind="ExternalInput")
    ones = nc.dram_tensor("ones", [1, 128], F32, kind="ExternalInput")
    probs = nc.dram_tensor("probs", [NSH, E], F32, kind="ExternalOutput")
    route = nc.dram_tensor("route", [NSH, 4], F32, kind="ExternalOutput")

    KC = D // 128  # 4 contraction chunks

    with tile.TileContext(nc) as tc:
        with (
            tc.tile_pool(name="const", bufs=1) as cpool,
            tc.tile_pool(name="xpool", bufs=D // 128 + 1) as xpool,
            tc.tile_pool(name="work", bufs=2) as wpool,
            tc.tile_pool(name="psum", bufs=2, space="PSUM") as ppool,
        ):
            gw_sb = cpool.tile([128, KC, E], F32)
            nc.sync.dma_start(gw_sb[:], gw.ap().rearrange("(c p) e -> p c e", p=128))
            gb_sb = cpool.tile([1, E], F32)
            nc.sync.dma_start(gb_sb[:], gb.ap())
            iota_sb = cpool.tile([128, E], F32)
            nc.sync.dma_start(iota_sb[:], iota.ap())
            ones_sb = cpool.tile([1, 128], F32)
            nc.sync.dma_start(ones_sb[:], ones.ap())

            def body():
                # load xt in 4 k-chunk tiles so matmuls start early
                xts = []
                for kc in range(KC):
                    xt_c = xpool.tile([128, NSH], F32, tag="xt")
                    nc.sync.dma_start(
                        xt_c[:],
                        xt.ap()[kc * 128:(kc + 1) * 128, :],
                    )
                    xts.append(xt_c)
                lg = ppool.tile([128, GT, E], F32, tag="lg")
                for ti in range(GT):
                    for kc in range(KC):
                        nc.tensor.matmul(
                            lg[:, ti, :],
                            xts[kc][:, ti * 128:(ti + 1) * 128],
                            gw_sb[:, kc, :],
                            start=(kc == 0),
                            stop=False,
                        )
                    nc.tensor.matmul(
                        lg[:, ti, :], ones_sb[:, :], gb_sb[:, :],
                        start=False, stop=True,
                    )

                ex = wpool.tile([128, GT, E], F32, tag="ex")
                nc.scalar.activation(ex[:], lg[:], AF.Exp)
                ssum = wpool.tile([128, GT], F32, tag="ssum")
                nc.vector.tensor_reduce(ssum[:], ex[:], AX.X, ALU.add)
                rsum = wpool.tile([128, GT], F32, tag="rsum")
                nc.vector.reciprocal(rsum[:], ssum[:])
                pr = wpool.tile([128, GT, E], F32, tag="pr")
                nc.vector.tensor_tensor(
                    pr[:], ex[:], rsum[:, :, None].broadcast_to([128, GT, E]),
                    ALU.mult,
                )
                nc.sync.dma_start(
                    probs.ap().rearrange("(g p) e -> p g e", p=128), pr[:]
                )

                m1 = wpool.tile([128, GT], F32, tag="m1")
                nc.vector.tensor_reduce(m1[:], pr[:], AX.X, ALU.max)
                mask1 = wpool.tile([128, GT, E], F32, tag="mask1")
                nc.vector.tensor_tensor(
                    mask1[:], pr[:], m1[:, :, None].broadcast_to([128, GT, E]),
                    ALU.is_ge,
                )
                # notm = 1 - mask1 ; masked = pr * notm kills the argmax slot
                notm = wpool.tile([128, GT, E], F32, tag="notm")
                nc.vector.tensor_scalar(
                    notm[:], mask1[:], -1.0, 1.0, ALU.mult, ALU.add
                )
                masked = wpool.tile([128, GT, E], F32, tag="masked")
                nc.vector.tensor_tensor(masked[:], pr[:], notm[:], ALU.mult)
                m2 = wpool.tile([128, GT], F32, tag="m2")
                nc.vector.tensor_reduce(m2[:], masked[:], AX.X, ALU.max)
                mask2 = wpool.tile([128, GT, E], F32, tag="mask2")
                nc.vector.tensor_tensor(
                    mask2[:], masked[:], m2[:, :, None].broadcast_to([128, GT, E]),
                    ALU.is_ge,
                )
                # arg indices via max(mask * iota)
                t1 = wpool.tile([128, GT, E], F32, tag="t1")
                nc.vector.tensor_tensor(
                    t1[:], mask1[:], iota_sb[:, None, :].broadcast_to([128, GT, E]),
                    ALU.mult,
                )
                t2 = wpool.tile([128, GT, E], F32, tag="t2")
                nc.vector.tensor_tensor(
                    t2[:], mask2[:], iota_sb[:, None, :].broadcast_to([128, GT, E]),
                    ALU.mult,
                )
                rt = wpool.tile([128, GT, 4], F32, tag="rt")
                nc.vector.tensor_reduce(rt[:, :, 0], t1[:], AX.X, ALU.max)
                nc.vector.tensor_reduce(rt[:, :, 1], t2[:], AX.X, ALU.max)
                # renormalized top-2 weights
                wsum = wpool.tile([128, GT], F32, tag="wsum")
                nc.vector.tensor_tensor(wsum[:], m1[:], m2[:], ALU.add)
                rw = wpool.tile([128, GT], F32, tag="rw")
                nc.vector.reciprocal(rw[:], wsum[:])
                nc.vector.tensor_tensor(rt[:, :, 2], m1[:], rw[:], ALU.mult)
                nc.vector.tensor_tensor(rt[:, :, 3], m2[:], rw[:], ALU.mult)
                nc.sync.dma_start(
                    route.ap().rearrange("(g p) f -> p g f", p=128), rt[:]
                )

            if reps == 1:
                body()
            else:
                with tc.For_i(0, reps, 1):
                    body()

    nc.compile()
    return nc


def _build_expert_nc(cap, act=AF.Gelu, reps=1):
    """Launch B: one expert's MLP over `cap` gathered token slots."""
    nc = bacc.Bacc("TRN2", target_bir_lowering=False, debug=False, num_devices=NC)
    xb = nc.dram_tensor("xb", [N, D], BF16, kind="ExternalInput")
    idx = nc.dram_tensor("idx", [128, cap // 16], I16, kind="ExternalInput")
    gat = nc.dram_tensor("gat", [128, cap // 128], F32, kind="ExternalInput")
    w1 = nc.dram_tensor("w1", [D, H], BF16, kind="ExternalInput")
    w2 = nc.dram_tensor("w2", [H, D], BF16, kind="ExternalInput")
    b1 = nc.dram_tensor("b1", [128, H // 128], F32, kind="ExternalInput")
    y = nc.dram_tensor("y", [cap, D], BF16, kind="ExternalOutput")

    KC = D // 128    # 4 k-chunks for layer 1
    MH = H // 128    # 8 h-tiles
    CH = 384         # gather/compute chunk (tokens); PSUM N<=512, %128==0
    assert cap % CH == 0
    GS = cap // CH   # number of chunks

    with tile.TileContext(nc) as tc:
        with (
            tc.tile_pool(name="wts", bufs=2) as wtpool,
            tc.tile_pool(name="xg", bufs=GS + 1) as xgpool,
            tc.tile_pool(name="h", bufs=3) as hpool,
            tc.tile_pool(name="y", bufs=3) as ypool,
            tc.tile_pool(name="ps1", bufs=4, space="PSUM") as ps1,
            tc.tile_pool(name="ps2", bufs=3, space="PSUM") as ps2,
        ):
            def body():
                idx_sb = wtpool.tile([128, cap // 16], I16, tag="idx")
                nc.sync.dma_start(idx_sb[:], idx.ap())
                gat_sb = wtpool.tile([128, cap // 128], F32, tag="gat")
                nc.sync.dma_start(gat_sb[:], gat.ap())
                w1_sb = wtpool.tile([128, KC, H], BF16, tag="w1")
                nc.sync.dma_start(
                    w1_sb[:], w1.ap().rearrange("(c p) h -> p c h", p=128)
                )
                w2_sb = wtpool.tile([128, MH, D], BF16, tag="w2")
                nc.sync.dma_start(
                    w2_sb[:], w2.ap().rearrange("(c p) d -> p c d", p=128)
                )
                b1_sb = wtpool.tile([128, MH], F32, tag="b1")
                nc.sync.dma_start(b1_sb[:], b1.ap())

                # Chunked gather of x rows, transposed on the fly:
                # xg_g[p, c, s] = x[idx[g*CH + s], c*128 + p].
                # Separate tiles per chunk: all gathers issue up front and
                # stream on the DMA engines while the PE computes.
                xgs = []
                for g in range(GS):
                    xg = xgpool.tile([128, KC, CH], BF16, tag="xg")
                    nc.gpsimd.dma_gather(
                        xg[:], xb.ap(),
                        idx_sb[:, g * (CH // 16):(g + 1) * (CH // 16)],
                        CH, CH, D, transpose=True, single_packet=False,
                    )
                    xgs.append(xg)

                for g in range(GS):
                    # Layer 1 on chunk g:
                    #   hT[m*128+p, s] = gelu(sum_d x[s,d] w1[d, m*128+p]+b1)
                    h_sb = hpool.tile([128, MH, CH], BF16, tag="h")
                    for m in range(MH):
                        ph = ps1.tile([128, CH], F32, tag="ph")
                        for kc in range(KC):
                            nc.tensor.matmul(
                                ph[:],
                                w1_sb[:, kc, m * 128:(m + 1) * 128],
                                xgs[g][:, kc, :],
                                start=(kc == 0),
                                stop=(kc == KC - 1),
                            )
                        nc.scalar.activation(
                            h_sb[:, m, :], ph[:], act, bias=b1_sb[:, m:m + 1]
                        )
                    # Layer 2 on chunk g:
                    #   y[t*128+p, :] = (hT[:, t*128+p] @ w2) * gating
                    # (b2 is added by the host combine instead.)
                    for tl in range(CH // 128):
                        t = g * (CH // 128) + tl
                        py = ps2.tile([128, D], F32, tag="py")
                        for hc in range(MH):
                            nc.tensor.matmul(
                                py[:],
                                h_sb[:, hc, tl * 128:(tl + 1) * 128],
                                w2_sb[:, hc, :],
                                start=(hc == 0),
                                stop=(hc == MH - 1),
                            )
                        y_t = ypool.tile([128, D], BF16, tag="yt")
                        nc.vector.tensor_scalar_mul(
                            y_t[:], py[:], gat_sb[:, t:t + 1]
                        )
                        nc.sync.dma_start(y.ap()[t * 128:(t + 1) * 128, :], y_t[:])

            if reps == 1:
                body()
            else:
                with tc.For_i(0, reps, 1):
                    body()

    nc.compile()
    return nc


def _gate_nc():
    if "gate" not in _CACHE:
        _CACHE["gate"] = _build_gate_nc()
    return _CACHE["gate"]


def _expert_nc(cap):
    key = ("expert", cap)
    if key not in _CACHE:
        _CACHE[key] = _build_expert_nc(cap)
    return _CACHE[key]


def _run(nc, in_maps, **kw):
    return bass_utils.run_bass_kernel_spmd(
        nc, in_maps, core_ids=list(range(NC)), **kw
    )


def _prep_gate_in_maps(xf, gate_w, gate_b):
    xT = np.ascontiguousarray(xf.T)                       # [D, N]
    ones = np.ones((1, 128), np.float32)
    iota = np.tile(np.arange(E, dtype=np.float32), (128, 1))
    return [
        {
            "xt": np.ascontiguousarray(xT[:, c * NSH:(c + 1) * NSH]),
            "gw": gate_w, "gb": gate_b, "iota": iota, "ones": ones,
        }
        for c in range(NC)
    ]


def _dispatch(route):
    """Device-computed routing records -> per-expert token/weight lists."""
    pair_tok = np.tile(np.arange(N, dtype=np.int64), 2)
    pair_exp = np.concatenate([route[:, 0], route[:, 1]]).astype(np.int64)
    pair_w = np.concatenate([route[:, 2], route[:, 3]]).astype(np.float32)
    toks, ws, cnts = [], [], []
    for e in range(E):
        sel = pair_exp == e
        toks.append(pair_tok[sel])
        ws.append(pair_w[sel])
        cnts.append(int(sel.sum()))
    cap = max(768, -(-max(cnts) // 384) * 384)
    return toks, ws, cnts, cap


def _prep_expert_in_maps(xf, w1, b1, w2, toks, ws, cnts, cap):
    xb = np.ascontiguousarray(xf.astype(ml_dtypes.bfloat16))
    in_maps = []
    for e in range(E):
        idx_arr = np.zeros(cap, np.int16)
        idx_arr[:cnts[e]] = toks[e].astype(np.int16)
        gat_arr = np.zeros(cap, np.float32)
        gat_arr[:cnts[e]] = ws[e]
        in_maps.append({
            "xb": xb,
            "idx": np.ascontiguousarray(np.tile(idx_arr.reshape(-1, 16).T, (8, 1))),
            "gat": np.ascontiguousarray(gat_arr.reshape(-1, 128).T),
            "w1": np.ascontiguousarray(w1[e].astype(ml_dtypes.bfloat16)),
            "w2": np.ascontiguousarray(w2[e].astype(ml_dtypes.bfloat16)),
            "b1": np.ascontiguousarray(b1[e].reshape(-1, 128).T),
        })
    return in_maps


def kernel(x, gate_w, gate_b, w1, b1, w2, b2):
    x = np.ascontiguousarray(np.asarray(x, np.float32))
    gate_w = np.ascontiguousarray(np.asarray(gate_w, np.float32))
    gate_b = np.ascontiguousarray(np.asarray(gate_b, np.float32)).reshape(1, E)
    w1 = np.asarray(w1, np.float32)
    b1 = np.asarray(b1, np.float32)
    w2 = np.asarray(w2, np.float32)
    b2 = np.asarray(b2, np.float32)

    xf = x.reshape(N, D)

    # ---- Launch A: gate ----
    resA = _run(_gate_nc(), _prep_gate_in_maps(xf, gate_w, gate_b))
    probs = np.concatenate([r["probs"] for r in resA.results], 0)  # [N, E]
    route = np.concatenate([r["route"] for r in resA.results], 0)  # [N, 4]

    # ---- Host dispatch (all-to-all sharding by routed expert) ----
    toks, ws, cnts, cap = _dispatch(route)

    # ---- Launch B: expert MLPs ----
    resB = _run(_expert_nc(cap),
                _prep_expert_in_maps(xf, w1, b1, w2, toks, ws, cnts, cap))

    # ---- Host combine (inverse of the dispatch permutation) ----
    # b2 was not added on device; each routed pair contributes w * b2[e].
    out = np.zeros((N, D), np.float32)
    for e in range(E):
        ye = resB.results[e]["y"][:cnts[e]].astype(np.float32)
        out[toks[e]] += ye + ws[e][:, None] * b2[e][None, :]

    return out.reshape(B, S, D), probs.reshape(B, S, E)
